# revision 8
# baseline (speedup 1.0000x reference)
"""Trainium2 Bass kernel for nn_Attention_47777216200735.

Module: q = (Xq @ Wq.T + bq) * D^-0.5 ; k = Xk @ Wk.T + bk
        out = softmax(q @ k.T, axis=keys) @ k    (per batch/head; V == K)

Shapes: B=4, S=2048, DQ=DK=1024, H=16, D=64, fp32.

Sharding (8 NeuronCores): core c = (b, g) with b = c//2 (batch, data
parallel) and g = c%2 (head-group, tensor parallel: heads g*8..g*8+7 and
the matching 512 rows of Wq/Wk). Attention is fully independent per
(b, h) so no collectives are needed; the host scatters inputs and
gathers/normalizes outputs.

Per-core graph (all layouts chosen so no on-chip input transposes exist;
the host ships Xq.T, Xk.T, Wq_shard.T, Wk_shard.T):
  1. qT[c,s], kT[c,s] = W.T-tile.T @ X.T  (f32r matmuls, full PE util)
  2. k_ext[h,j] = PE-transpose of kT blocks -> [s,d] natural layout,
     cast bf16, plus a ones column (col 64)
  3. per head: scoresT[j,i] = kT_h.T @ qT_h (f32r); exp via ScalarE
     (scale=1/8 folded into the activation's free affine, bf16 out, no
     max-subtraction -- logits are provably O(10));
     out[i, 0:65] += expT[:,i-tile].T @ k_ext  accumulated over j in
     PSUM; column 64 accumulates the softmax denominator for free.
  4. DMA the unnormalized [i, 65] tiles out; the host divides.
"""

import numpy as np
from contextlib import ExitStack

import concourse.bass as bass
import concourse.bacc as bacc
import concourse.tile as tile
import concourse.mybir as mybir
from concourse.bass_utils import run_bass_kernel_spmd
F32 = mybir.dt.float32
F32R = mybir.dt.float32r
BF16 = mybir.dt.bfloat16
EXP = mybir.ActivationFunctionType.Exp

B, S, DQ, H, D = 4, 2048, 1024, 16, 64
P = 128
HC = H // 2          # heads per core = 8
C = HC * D           # projection channels per core = 512
KT = DQ // P         # 8 contraction tiles
CT = C // P          # 4 channel tiles (2 heads each)
NJ = S // P          # 16 key tiles
IC = 1024            # query chunk (softmax/psum blocking)
NIC = S // IC        # 2
SCALE = float(D) ** -0.5

_CACHE: dict = {}


def _build(has_bias: bool):
    nc = bacc.Bacc("TRN2", target_bir_lowering=False, debug=False)

    xqt = nc.dram_tensor("xqt", [DQ, S], F32R, kind="ExternalInput").ap()
    xkt = nc.dram_tensor("xkt", [DQ, S], F32R, kind="ExternalInput").ap()
    wqt = nc.dram_tensor("wqt", [DQ, C], F32R, kind="ExternalInput").ap()
    wkt = nc.dram_tensor("wkt", [DQ, C], F32R, kind="ExternalInput").ap()
    if has_bias:
        bqr = nc.dram_tensor("bqr", [1, C], F32R, kind="ExternalInput").ap()
        bkr = nc.dram_tensor("bkr", [1, C], F32R, kind="ExternalInput").ap()
        onesd = nc.dram_tensor("onesd", [1, S], F32R, kind="ExternalInput").ap()
    idn = nc.dram_tensor("idn", [P, P], F32R, kind="ExternalInput").ap()
    out = nc.dram_tensor("out", [HC, S, D + 1], F32, kind="ExternalOutput").ap()

    with tile.TileContext(nc) as tc, ExitStack() as ctx:
        const_p = ctx.enter_context(tc.tile_pool(name="const", bufs=1))
        w_p = ctx.enter_context(tc.tile_pool(name="wp", bufs=2 * KT))
        x_p = ctx.enter_context(tc.tile_pool(name="xp", bufs=KT))
        qk_p = ctx.enter_context(tc.tile_pool(name="qkp", bufs=CT))
        kext_p = ctx.enter_context(tc.tile_pool(name="kextp", bufs=1))
        exp_p = ctx.enter_context(tc.tile_pool(name="expp", bufs=3))
        ob_p = ctx.enter_context(tc.tile_pool(name="obp", bufs=2))
        psumA = ctx.enter_context(tc.tile_pool(name="psA", bufs=2, space="PSUM"))
        psumB = ctx.enter_context(tc.tile_pool(name="psB", bufs=4, space="PSUM"))

        ident = const_p.tile([P, P], F32R)
        nc.sync.dma_start(out=ident[:], in_=idn[:])

        # k_ext: one big tile, slices (h, j) -> [128 keys, 64 d + ones]
        kext = kext_p.tile([P, HC * NJ * (D + 1)], BF16)
        nc.gpsimd.memset(kext[:], 1.0)

        def kx(h, j):
            o = (h * NJ + j) * (D + 1)
            return kext[:, o:o + D + 1]

        if has_bias:
            ones_sb = const_p.tile([1, S], F32R)
            nc.sync.dma_start(out=ones_sb[:], in_=onesd[:])
            bq_sb = const_p.tile([1, C], F32R)
            bk_sb = const_p.tile([1, C], F32R)
            nc.sync.dma_start(out=bq_sb[:], in_=bqr[:])
            nc.sync.dma_start(out=bk_sb[:], in_=bkr[:])

        # ---- weights ----
        w_tiles = {}
        for name, src in (("q", wqt), ("k", wkt)):
            for kt in range(KT):
                t = w_p.tile([P, C], F32R, tag="w")
                nc.sync.dma_start(out=t[:], in_=src[kt * P:(kt + 1) * P, :])
                w_tiles[name, kt] = t

        # ---- projections: proj[c-tile, s] accumulated over k-tiles ----
        qk_tiles = {}

        def project(name, src_x, bias_sb):
            xt = []
            for kt in range(KT):
                t = x_p.tile([P, S], F32R, tag="x")
                nc.sync.dma_start(out=t[:], in_=src_x[kt * P:(kt + 1) * P, :])
                xt.append(t)
            for ct in range(CT):
                dst = qk_p.tile([P, S], F32R, tag=f"qk_{name}", name=f"{name}T{ct}")
                for sb in range(S // 512):
                    ps = psumA.tile([P, 512], F32, tag="big")
                    n_acc = KT + (1 if has_bias else 0)
                    for kt in range(KT):
                        nc.tensor.matmul(
                            ps[:],
                            lhsT=w_tiles[name, kt][:, ct * P:(ct + 1) * P],
                            rhs=xt[kt][:, sb * 512:(sb + 1) * 512],
                            start=(kt == 0),
                            stop=(kt == n_acc - 1),
                        )
                    if has_bias:
                        nc.tensor.matmul(
                            ps[:],
                            lhsT=bias_sb[:, ct * P:(ct + 1) * P],
                            rhs=ones_sb[:, sb * 512:(sb + 1) * 512],
                            start=False,
                            stop=True,
                        )
                    nc.vector.tensor_copy(dst[:, sb * 512:(sb + 1) * 512], ps[:])
                qk_tiles[name, ct] = dst

        project("q", xqt, bq_sb if has_bias else None)
        project("k", xkt, bk_sb if has_bias else None)

        # ---- k natural layout via PE transpose of kT blocks ----
        for ct in range(CT):
            for j in range(NJ):
                tp = psumB.tile([P, P], F32R, tag="acc")
                nc.tensor.transpose(
                    tp[:], qk_tiles["k", ct][:, j * P:(j + 1) * P], ident[:]
                )
                nc.vector.tensor_copy(kx(2 * ct, j)[:, 0:D], tp[:, 0:D])
                nc.vector.tensor_copy(kx(2 * ct + 1, j)[:, 0:D], tp[:, D:P])

        # ---- attention ----
        for h in range(HC):
            ct, e = divmod(h, 2)
            qTh = qk_tiles["q", ct][e * D:(e + 1) * D, :]
            kTh = qk_tiles["k", ct][e * D:(e + 1) * D, :]
            for ic in range(NIC):
                accs = [psumB.tile([P, 4 * (D + 1)], F32, tag="acc",
                                   name=f"acc{h}_{ic}_{a}")
                        for a in range(IC // (4 * P))]
                for j in range(NJ):
                    sp = psumA.tile([P, IC], F32, tag="big")
                    for u in range(IC // 512):
                        nc.tensor.matmul(
                            sp[:, u * 512:(u + 1) * 512],
                            lhsT=kTh[:, j * P:(j + 1) * P],
                            rhs=qTh[:, ic * IC + u * 512: ic * IC + (u + 1) * 512],
                            start=True,
                            stop=True,
                        )
                    et = exp_p.tile([P, IC], BF16, tag="exp")
                    nc.scalar.activation(et[:], sp[:], EXP, scale=SCALE)
                    for t in range(IC // P):
                        # start=True resets PSUM has_written BANK-wide, so
                        # only the first slice of each acc bank may issue it;
                        # later slices land on cleared has_written bits and
                        # overwrite stale data on their first matmul.
                        sl = accs[t // 4][:, (t % 4) * (D + 1):(t % 4 + 1) * (D + 1)]
                        nc.tensor.matmul(
                            sl,
                            lhsT=et[:, t * P:(t + 1) * P],
                            rhs=kx(h, j)[:],
                            start=(j == 0 and t % 4 == 0),
                            stop=(j == NJ - 1),
                            skip_group_check=True,
                        )
                for gi, acc in enumerate(accs):
                    ob = ob_p.tile([P, 4 * (D + 1)], F32, tag="ob")
                    nc.vector.tensor_copy(ob[:], acc[:])
                    for u in range(4):
                        r0 = ic * IC + gi * 512 + u * P
                        nc.sync.dma_start(
                            out=out[h, r0:r0 + P, :],
                            in_=ob[:, u * (D + 1):(u + 1) * (D + 1)],
                        )

    nc.compile()
    return nc


def _transposed(x):
    return np.ascontiguousarray(np.asarray(x, dtype=np.float32).T)


def kernel(query_input, key_input, Wq, bq, Wk, bk):
    query_input = np.asarray(query_input, dtype=np.float32)
    key_input = np.asarray(key_input, dtype=np.float32)
    Wq = np.asarray(Wq, dtype=np.float32)
    Wk = np.asarray(Wk, dtype=np.float32)
    bq = np.asarray(bq, dtype=np.float32)
    bk = np.asarray(bk, dtype=np.float32)

    has_bias = bool(np.any(bq) or np.any(bk))
    if ("nc", has_bias) not in _CACHE:
        _CACHE["nc", has_bias] = _build(has_bias)
    nc = _CACHE["nc", has_bias]

    in_maps = []
    for c in range(8):
        b, g = divmod(c, 2)
        rows = slice(g * C, (g + 1) * C)
        m = {
            "idn": np.eye(P, dtype=np.float32),
            "xqt": _transposed(query_input[b]),
            "xkt": _transposed(key_input[b]),
            "wqt": _transposed(Wq[rows]),
            "wkt": _transposed(Wk[rows]),
        }
        if has_bias:
            m["bqr"] = np.ascontiguousarray(bq[rows])[None, :]
            m["bkr"] = np.ascontiguousarray(bk[rows])[None, :]
            m["onesd"] = np.ones((1, S), dtype=np.float32)
        in_maps.append(m)

    global _last_in_maps
    _last_in_maps = in_maps
    res = run_bass_kernel_spmd(nc, in_maps, core_ids=list(range(8)))

    full = np.empty((B, S, H * D), dtype=np.float32)
    for c in range(8):
        b, g = divmod(c, 2)
        o = res.results[c]["out"]                   # [HC, S, D+1]
        o = o[:, :, :D] / o[:, :, D:D + 1]          # softmax normalization
        full[b, :, g * C:(g + 1) * C] = o.transpose(1, 0, 2).reshape(S, C)
    return full


# revision 10
# speedup vs baseline: 1.1114x; 1.1114x over previous
"""Trainium2 Bass kernel for nn_Attention_47777216200735.

Module: q = (Xq @ Wq.T + bq) * D^-0.5 ; k = Xk @ Wk.T + bk
        out = softmax(q @ k.T, axis=keys) @ k    (per batch/head; V == K)

Shapes: B=4, S=2048, DQ=DK=1024, H=16, D=64, fp32.

Sharding (8 NeuronCores): core c = (b, g) with b = c//2 (batch, data
parallel) and g = c%2 (head-group, tensor parallel: heads g*8..g*8+7 and
the matching 512 rows of Wq/Wk). Attention is fully independent per
(b, h) so no collectives are needed; the host scatters inputs and
gathers/normalizes/transposes outputs.

Per-core graph (layouts chosen so no on-chip input transposes exist; the
host ships Xq.T, Xk.T, Wq_shard.T, Wk_shard.T):
  1. qT[c,s], kT[c,s] = W.T-tile.T @ X.T: f32r matmuls (K=128 f32r runs
     1 cyc/row), evicted to SBUF as bf16 for the attention matmuls.
  2. k_ext[h,j] = PE-transpose of kT blocks -> [s,d] natural layout
     (bf16) plus a ones column (col 64).
  3. per head, per 1024-query chunk, per 128-key tile j:
       scoresT[j,i] = kT_h(j).T @ qT_h  (bf16, full-speed at K=64)
       expT = Exp(scale * scoresT)      (ScalarE, scale=1/8 free affine,
                                         no max-subtraction: logits O(6))
       out'[d+1, i] += k_ext[h,j].T @ expT   (bf16; k_ext is the
         STATIONARY operand so the per-matmul LDWEIGHTS is tiny; the
         ones column accumulates the softmax denominator in row 64)
     Each out' PSUM bank holds exactly one accumulation group.
  4. DMA out' in [d+1, s] layout; the host divides by row 64 and
     transposes (free on host).
"""

import numpy as np
from contextlib import ExitStack

import concourse.bass as bass
import concourse.bacc as bacc
import concourse.tile as tile
import concourse.mybir as mybir
from concourse.bass_utils import run_bass_kernel_spmd

F32 = mybir.dt.float32
F32R = mybir.dt.float32r
BF16 = mybir.dt.bfloat16
EXP = mybir.ActivationFunctionType.Exp

B, S, DQ, H, D = 4, 2048, 1024, 16, 64
P = 128
HC = H // 2          # heads per core = 8
C = HC * D           # projection channels per core = 512
KT = DQ // P         # 8 contraction tiles
CT = C // P          # 4 channel tiles (2 heads each)
NJ = S // P          # 16 key tiles
IC = 1024            # query chunk (softmax/psum blocking)
NIC = S // IC        # 2
SCALE = float(D) ** -0.5

_CACHE: dict = {}
_last_in_maps = None


def _build(has_bias: bool):
    nc = bacc.Bacc("TRN2", target_bir_lowering=False, debug=False)

    xqt = nc.dram_tensor("xqt", [DQ, S], F32R, kind="ExternalInput").ap()
    xkt = nc.dram_tensor("xkt", [DQ, S], F32R, kind="ExternalInput").ap()
    wqt = nc.dram_tensor("wqt", [DQ, C], F32R, kind="ExternalInput").ap()
    wkt = nc.dram_tensor("wkt", [DQ, C], F32R, kind="ExternalInput").ap()
    if has_bias:
        bqr = nc.dram_tensor("bqr", [1, C], F32R, kind="ExternalInput").ap()
        bkr = nc.dram_tensor("bkr", [1, C], F32R, kind="ExternalInput").ap()
        onesd = nc.dram_tensor("onesd", [1, S], F32R, kind="ExternalInput").ap()
    idn = nc.dram_tensor("idn", [P, P], F32, kind="ExternalInput").ap()
    out = nc.dram_tensor("out", [HC, D + 1, S], F32, kind="ExternalOutput").ap()

    with tile.TileContext(nc) as tc, ExitStack() as ctx:
        const_p = ctx.enter_context(tc.tile_pool(name="const", bufs=1))
        w_p = ctx.enter_context(tc.tile_pool(name="wp", bufs=2 * KT))
        x_p = ctx.enter_context(tc.tile_pool(name="xp", bufs=KT))
        qk_p = ctx.enter_context(tc.tile_pool(name="qkp", bufs=CT))
        kext_p = ctx.enter_context(tc.tile_pool(name="kextp", bufs=1))
        exp_p = ctx.enter_context(tc.tile_pool(name="expp", bufs=3))
        ob_p = ctx.enter_context(tc.tile_pool(name="obp", bufs=2))
        # PSUM: scores 2 slots x [128,1024]f32 (2 banks each) = 4 banks;
        # out' accumulators / kext-transpose tiles share 2 slots x 2 banks.
        psumA = ctx.enter_context(tc.tile_pool(name="psA", bufs=2, space="PSUM"))
        psumB = ctx.enter_context(tc.tile_pool(name="psB", bufs=2, space="PSUM"))

        identf = const_p.tile([P, P], F32)
        nc.sync.dma_start(out=identf[:], in_=idn[:])
        identb = const_p.tile([P, P], BF16)
        nc.vector.tensor_copy(identb[:], identf[:])

        # k_ext: one big tile, slices (h, j) -> [128 keys, 64 d + ones]
        kext = kext_p.tile([P, HC * NJ * (D + 1)], BF16)
        nc.gpsimd.memset(kext[:], 1.0)

        def kx(h, j):
            o = (h * NJ + j) * (D + 1)
            return kext[:, o:o + D + 1]

        if has_bias:
            ones_sb = const_p.tile([1, S], F32R)
            nc.sync.dma_start(out=ones_sb[:], in_=onesd[:])
            bq_sb = const_p.tile([1, C], F32R)
            bk_sb = const_p.tile([1, C], F32R)
            nc.sync.dma_start(out=bq_sb[:], in_=bqr[:])
            nc.sync.dma_start(out=bk_sb[:], in_=bkr[:])

        # ---- weights ----
        w_tiles = {}
        for name, src in (("q", wqt), ("k", wkt)):
            for kt in range(KT):
                t = w_p.tile([P, C], F32R, tag="w", name=f"w{name}{kt}")
                nc.sync.dma_start(out=t[:], in_=src[kt * P:(kt + 1) * P, :])
                w_tiles[name, kt] = t

        qk_tiles = {}

        def load_x(name, src_x):
            xt = []
            for kt in range(KT):
                t = x_p.tile([P, S], F32R, tag="x", name=f"x{name}{kt}")
                nc.sync.dma_start(out=t[:], in_=src_x[kt * P:(kt + 1) * P, :])
                xt.append(t)
            return xt

        def project(name, xt, bias_sb, cts):
            """proj[c-tile, s] over k-tiles; evict psum f32 -> sbuf bf16."""
            for ct in cts:
                dst = qk_p.tile([P, S], BF16, tag=f"qk_{name}", name=f"{name}T{ct}")
                for sb in range(S // 512):
                    ps = psumA.tile([P, 512], F32, tag="big", name=f"ps{name}{ct}{sb}")
                    n_acc = KT + (1 if has_bias else 0)
                    for kt in range(KT):
                        nc.tensor.matmul(
                            ps[:],
                            lhsT=w_tiles[name, kt][:, ct * P:(ct + 1) * P],
                            rhs=xt[kt][:, sb * 512:(sb + 1) * 512],
                            start=(kt == 0),
                            stop=(kt == n_acc - 1),
                        )
                    if has_bias:
                        nc.tensor.matmul(
                            ps[:],
                            lhsT=bias_sb[:, ct * P:(ct + 1) * P],
                            rhs=ones_sb[:, sb * 512:(sb + 1) * 512],
                            start=False,
                            stop=True,
                        )
                    nc.vector.tensor_copy(dst[:, sb * 512:(sb + 1) * 512], ps[:])
                qk_tiles[name, ct] = dst

        def kext_build(ct):
            for j in range(NJ):
                tp = psumB.tile([P, P], BF16, tag="acc", name=f"tp{ct}_{j}")
                nc.tensor.transpose(
                    tp[:], qk_tiles["k", ct][:, j * P:(j + 1) * P], identb[:]
                )
                nc.vector.tensor_copy(kx(2 * ct, j)[:, 0:D], tp[:, 0:D])
                nc.vector.tensor_copy(kx(2 * ct + 1, j)[:, 0:D], tp[:, D:P])

        def attention(h):
            ct, e = divmod(h, 2)
            qTh = qk_tiles["q", ct][e * D:(e + 1) * D, :]
            kTh = qk_tiles["k", ct][e * D:(e + 1) * D, :]
            for ic in range(NIC):
                acc = psumB.tile([D + 1, IC], F32, tag="acc", name=f"acc{h}_{ic}")
                for j in range(NJ):
                    sp = psumA.tile([P, IC], F32, tag="big", name=f"sp{h}{ic}{j}")
                    for u in range(IC // 512):
                        nc.tensor.matmul(
                            sp[:, u * 512:(u + 1) * 512],
                            lhsT=kTh[:, j * P:(j + 1) * P],
                            rhs=qTh[:, ic * IC + u * 512: ic * IC + (u + 1) * 512],
                            start=True,
                            stop=True,
                        )
                    et = exp_p.tile([P, IC], BF16, tag="exp", name=f"et{h}{ic}{j}")
                    nc.scalar.activation(et[:], sp[:], EXP, scale=SCALE)
                    # out'[d+1, i] += k_ext[h,j].T @ expT ; one accumulation
                    # group per PSUM bank (512-col halves of acc).
                    for u in range(IC // 512):
                        nc.tensor.matmul(
                            acc[:, u * 512:(u + 1) * 512],
                            lhsT=kx(h, j)[:],
                            rhs=et[:, u * 512:(u + 1) * 512],
                            start=(j == 0),
                            stop=(j == NJ - 1),
                        )
                ob = ob_p.tile([D + 1, IC], F32, tag="ob", name=f"ob{h}_{ic}")
                nc.vector.tensor_copy(ob[:], acc[:])
                nc.sync.dma_start(
                    out=out[h, :, ic * IC:(ic + 1) * IC], in_=ob[:]
                )

        # ---- emission order chosen for PE/ACT overlap: k-side first, then
        # q-projection per channel-tile immediately followed by its heads'
        # attention, so later q-proj matmuls overlap earlier heads' exps.
        xk = load_x("k", xkt)
        project("k", xk, bk_sb if has_bias else None, range(CT))
        for ct in range(CT):
            kext_build(ct)
        xq = load_x("q", xqt)
        for ct in range(CT):
            project("q", xq, bq_sb if has_bias else None, [ct])
            attention(2 * ct)
            attention(2 * ct + 1)

    nc.compile()
    return nc


def _transposed(x):
    return np.ascontiguousarray(np.asarray(x, dtype=np.float32).T)


def kernel(query_input, key_input, Wq, bq, Wk, bk):
    query_input = np.asarray(query_input, dtype=np.float32)
    key_input = np.asarray(key_input, dtype=np.float32)
    Wq = np.asarray(Wq, dtype=np.float32)
    Wk = np.asarray(Wk, dtype=np.float32)
    bq = np.asarray(bq, dtype=np.float32)
    bk = np.asarray(bk, dtype=np.float32)

    has_bias = bool(np.any(bq) or np.any(bk))
    if ("nc", has_bias) not in _CACHE:
        _CACHE["nc", has_bias] = _build(has_bias)
    nc = _CACHE["nc", has_bias]

    in_maps = []
    for c in range(8):
        b, g = divmod(c, 2)
        rows = slice(g * C, (g + 1) * C)
        m = {
            "idn": np.eye(P, dtype=np.float32),
            "xqt": _transposed(query_input[b]),
            "xkt": _transposed(key_input[b]),
            "wqt": _transposed(Wq[rows]),
            "wkt": _transposed(Wk[rows]),
        }
        if has_bias:
            m["bqr"] = np.ascontiguousarray(bq[rows])[None, :]
            m["bkr"] = np.ascontiguousarray(bk[rows])[None, :]
            m["onesd"] = np.ones((1, S), dtype=np.float32)
        in_maps.append(m)

    global _last_in_maps
    _last_in_maps = in_maps
    res = run_bass_kernel_spmd(nc, in_maps, core_ids=list(range(8)))

    full = np.empty((B, S, H * D), dtype=np.float32)
    for c in range(8):
        b, g = divmod(c, 2)
        o = res.results[c]["out"]                    # [HC, D+1, S]
        o = o[:, :D, :] / o[:, D:D + 1, :]           # softmax normalization
        full[b, :, g * C:(g + 1) * C] = o.transpose(2, 0, 1).reshape(S, C)
    return full
